# revision 1
# baseline (speedup 1.0000x reference)
"""Trainium2 Bass kernel for nn_Attention_42348377538911.

3D attention: x [2, 128, 16, 16, 16] -> qkv 1x1x1 conv -> 4-head attention
over N=4096 positions (dim_head=32) -> out 1x1x1 conv.

Sharding: 8 cores = 2 batches x 4 heads (one (b, h) pair per core).
Each core computes its head's attention and a tensor-parallel partial of the
output projection (w_out split along hidden); host sums the 4 partials per
batch and adds b_out.

Per-core kernel layout (all attention math in simT = [j, i] orientation so no
transposes are ever needed):
  qk-proj : psum[64, 512] = w_qkT.T @ x-tile          (q rows scaled by d^-1/2)
  vT-proj : psum[128, 32] = x-chunk.T @ w_vT          (vT directly, x stationary)
  simT    : psum[128j, 512i] = k-chunk.T @ q-tile     (f32r, 4x row-packed K=32)
  exp     : ACT reads 4(3)-bank psum group, writes SBUF f32r
  AV+sums : psum[33, 512] += vT_aug-chunk.T @ expT    (col 32 of vT_aug = ones
                                                       -> row 32 = softmax denom)
  norm    : recip of sums (partition-0 hop) -> gpsimd partition_broadcast ->
            DVE multiply
  y-proj  : psum[128, 512] = w_oT.T @ out_normT; copy to SBUF; DMA out

PSUM budget: qkA 4 banks + qkB 3 banks (alternating exp groups, double
buffered against each other) + av 1 bank = 8.  y-proj borrows the qkB slot.
"""

import sys

import numpy as np

if "/opt/trn_rl_repo" not in sys.path:
    sys.path.insert(0, "/opt/trn_rl_repo")

HEADS = 4
DIM_HEAD = 32
B = 2
C = 128
N = 4096          # 16*16*16 spatial positions
NT = 512          # i-tile width
N_IT = N // NT    # 8 i-tiles
A_GROUPS = 5      # chunks 7g .. 7g+4   (4-wide, 4 psum banks)
B_GROUPS = 4      # chunks 7g+4 .. 7g+7 (3-wide, 3 psum banks)

_cached = {}


def _build(reps=1):
    import concourse.bacc as bacc
    import concourse.tile as tile
    import concourse.mybir as mybir
    from concourse.bass import ts

    f32 = mybir.dt.float32
    f32r = mybir.dt.float32r
    EXP = mybir.ActivationFunctionType.Exp

    nc = bacc.Bacc("TRN2", target_bir_lowering=False, debug=False, num_devices=8)
    x_d = nc.dram_tensor("x", [C, N], f32, kind="ExternalInput").ap()
    w4q_d = nc.dram_tensor("w_4q", [C, C], f32, kind="ExternalInput").ap()
    w4k_d = nc.dram_tensor("w_4k", [C, C], f32, kind="ExternalInput").ap()
    wvt_d = nc.dram_tensor("w_vT", [C, DIM_HEAD], f32, kind="ExternalInput").ap()
    wot_d = nc.dram_tensor("w_oT", [DIM_HEAD, C], f32, kind="ExternalInput").ap()
    y_d = nc.dram_tensor("y", [C, N], f32, kind="ExternalOutput").ap()

    # processing order per i-tile: A0 B0 A1 B1 A2 B2 A3 B3 A4
    seq = []
    for g in range(A_GROUPS):
        seq.append(("A", g, 7 * g, 4))
        if g < B_GROUPS:
            seq.append(("B", g, 7 * g + 4, 3))

    with tile.TileContext(nc) as tc:
        with tc.tile_pool(name="sing", bufs=1) as sing:
            # long-lived SBUF tensors
            w4q = sing.tile([C, C], f32r)
            w4k = sing.tile([C, C], f32r)
            wvt = sing.tile([C, DIM_HEAD], f32r)
            wot = sing.tile([DIM_HEAD, C], f32r)
            x_sb = [sing.tile([C, NT], f32r, tag=f"x{cx}", name=f"x{cx}")
                    for cx in range(8)]
            # q replicated at 4 partition bases, one tile per i-tile and
            # one k tile per group so QK deps are tile-granular
            q_rt = [sing.tile([128, NT], f32r, tag=f"qrt{it}", name=f"qrt{it}")
                    for it in range(N_IT)]
            k_rt = [sing.tile([128, 128], f32r, tag=f"krt{kg}", name=f"krt{kg}")
                    for kg in range(9)]
            vt_aug = sing.tile([128, 32, 33], f32r)     # per chunk [j, d + ones]
            scr = sing.tile([1, 64], f32)

            nc.sync.dma_start(w4q, w4q_d.bitcast(f32r))
            nc.sync.dma_start(w4k, w4k_d.bitcast(f32r))
            nc.sync.dma_start(wvt, wvt_d.bitcast(f32r))
            nc.sync.dma_start(wot, wot_d.bitcast(f32r))
            # x in 8 chunks so the first projection starts after 512 cols;
            # issue these before the exp-table warm-up so its ~2.7us
            # ACT_TABLE_LOAD doesn't block the x0 issue on the scalar queue
            for cx in range(8):
                nc.scalar.dma_start(x_sb[cx], x_d[:, ts(cx, NT)].bitcast(f32r))
            # warm the ACT exp table set while P0 runs
            nc.vector.memset(scr, 0.0)
            nc.scalar.activation(scr, scr, EXP)
            nc.vector.memset(vt_aug[:].bitcast(f32), 1.0)

            for rep in range(reps):
                # ------- P0: replicated projections (no layout DMAs) -------
                # chunk jc -> (group index, row) in processing layout
                chunk_pos = {}
                for kind, g, jc0, width in seq:
                    kg = g if kind == "A" else A_GROUPS + g
                    for r in range(width):
                        chunk_pos[jc0 + r] = (kg, r)

                with tc.tile_pool(name="p0ps", bufs=2, space="PSUM") as p0ps:
                    if rep == 0:
                        warm = p0ps.tile([128, NT], f32, tag="pq")
                        for _ in range(7):
                            nc.tensor.matmul(warm[:, 0:C], lhsT=w4q, rhs=w4q,
                                             start=True, stop=True)
                    for it in range(N_IT):
                        xs = x_sb[it]
                        psq = p0ps.tile([128, NT], f32, tag="pq")
                        nc.tensor.matmul(psq, lhsT=w4q, rhs=xs,
                                         start=True, stop=True)
                        nc.vector.tensor_copy(q_rt[it], psq)
                        psk = p0ps.tile([128, NT], f32, tag="pk")
                        nc.tensor.matmul(psk, lhsT=w4k, rhs=xs,
                                         start=True, stop=True)
                        for jj in range(4):
                            kg, r = chunk_pos[4 * it + jj]
                            nc.vector.tensor_copy(
                                k_rt[kg][32 * r:32 * r + 32, :],
                                psk[32 * r:32 * r + 32, ts(jj, 128)])

                # ---------------- P1: attention ----------------
                with tc.tile_pool(name="exA", bufs=10) as exA_pool, \
                     tc.tile_pool(name="exB", bufs=9) as exB_pool, \
                     tc.tile_pool(name="nrm", bufs=2) as nrm, \
                     tc.tile_pool(name="ysb", bufs=2) as ysb, \
                     tc.tile_pool(name="qkA", bufs=1, space="PSUM") as qkA, \
                     tc.tile_pool(name="qkB", bufs=1, space="PSUM") as qkB, \
                     tc.tile_pool(name="avp", bufs=1, space="PSUM") as avp:

                    ex_tiles = [None] * (N_IT + 1)
                    for step in range(N_IT + 1):
                        # QK + exp for i-tile `step`
                        if step < N_IT:
                            cur = []
                            for kind, g, jc0, width in seq:
                                if kind == "A":
                                    qk_ps = qkA.tile([128, 4 * NT], f32,
                                                     tag="qkA")
                                    ex_t = exA_pool.tile([128, 4 * NT], f32r,
                                                         tag="exA")
                                    kg = g
                                else:
                                    qk_ps = qkB.tile([128, 3 * NT], f32,
                                                     tag="qkB")
                                    ex_t = exB_pool.tile([128, 3 * NT], f32r,
                                                         tag="exB")
                                    kg = A_GROUPS + g
                                for r in range(width):
                                    nc.tensor.matmul(
                                        qk_ps[:, ts(r, NT)],
                                        lhsT=k_rt[kg][32 * r:32 * r + 32, :],
                                        rhs=q_rt[step][32 * r:32 * r + 32, :],
                                        start=True, stop=True,
                                        tile_position=(32 * r, 0))
                                nc.scalar.activation(ex_t, qk_ps, EXP)
                                cur.append((ex_t, jc0, width))
                            ex_tiles[step] = cur

                        # vT projection during step 0, borrowing the av bank
                        if step == 0:
                            for half in range(2):
                                ps2 = avp.tile([128, 512], f32, tag="avy")
                                for jj in range(16):
                                    jc = half * 16 + jj
                                    nc.tensor.matmul(
                                        ps2[:, ts(jj, 32)],
                                        lhsT=x_sb[jc // 4][:, ts(jc % 4, 128)],
                                        rhs=wvt,
                                        start=True, stop=True)
                                nc.vector.tensor_copy(
                                    vt_aug[:, half * 16:(half + 1) * 16,
                                           0:DIM_HEAD],
                                    ps2[:].rearrange("p (c d) -> p c d",
                                                     d=DIM_HEAD))

                        # AV + normalize + y for i-tile `step - 1`
                        if step > 0:
                            it = step - 1
                            av_ps = avp.tile([33, NT], f32, tag="avy")
                            n_mm = 0
                            for ex_t, jc0, width in ex_tiles[it]:
                                for r in range(width):
                                    nc.tensor.matmul(
                                        av_ps,
                                        lhsT=vt_aug[:, jc0 + r, :],
                                        rhs=ex_t[:, ts(r, NT)],
                                        start=(n_mm == 0), stop=(n_mm == 31))
                                    n_mm += 1
                            ex_tiles[it] = None

                            # single copy evacuates av (data + sums row); the
                            # av bank frees for the next i-tile immediately
                            on_raw = nrm.tile([33, NT], f32r, tag="onr")
                            nc.vector.tensor_copy(on_raw, av_ps)

                            # recip chain (concurrent with y matmul):
                            # sums row -> partition 0 -> recip -> bcast x128
                            t_s0 = nrm.tile([1, NT], f32, tag="ts0")
                            nc.sync.dma_start(t_s0, on_raw[32:33, :].bitcast(f32))
                            t_rc = nrm.tile([1, NT], f32, tag="trc")
                            nc.vector.reciprocal(t_rc, t_s0)
                            t_rcb = nrm.tile([128, NT], f32, tag="trcb")
                            nc.gpsimd.partition_broadcast(t_rcb, t_rc,
                                                          channels=128)

                            # y projection on unnormalized rows (linear in the
                            # per-column scale), borrows the qkB psum slot
                            y_ps = qkB.tile([128, NT], f32, tag="qkB")
                            nc.tensor.matmul(y_ps, lhsT=wot,
                                             rhs=on_raw[0:32, :],
                                             start=True, stop=True)
                            y_sb = ysb.tile([128, NT], f32, tag="ysb")
                            nc.vector.tensor_mul(y_sb, y_ps, t_rcb)
                            nc.sync.dma_start(y_d[:, ts(it, NT)], y_sb)

    nc.compile()
    return nc


def _get_nc():
    if "nc" not in _cached:
        _cached["nc"] = _build()
    return _cached["nc"]


def _make_in_maps(x, w_qkv, w_out):
    scale = DIM_HEAD ** -0.5
    in_maps = []
    for core in range(8):
        b, h = core // HEADS, core % HEADS
        w_q = w_qkv[h * DIM_HEAD:(h + 1) * DIM_HEAD, :]
        w_k = w_qkv[128 + h * DIM_HEAD:128 + (h + 1) * DIM_HEAD, :]
        w_v = w_qkv[256 + h * DIM_HEAD:256 + (h + 1) * DIM_HEAD, :]
        in_maps.append({
            "x": np.ascontiguousarray(x[b].reshape(C, N)),
            "w_4q": np.ascontiguousarray(np.tile(w_q.T * scale, (1, 4))),
            "w_4k": np.ascontiguousarray(np.tile(w_k.T, (1, 4))),
            "w_vT": np.ascontiguousarray(w_v.T),
            "w_oT": np.ascontiguousarray(
            w_out[:, h * DIM_HEAD:(h + 1) * DIM_HEAD].T),
        })
    return in_maps


def _gather(results, b_out):
    y = np.zeros((B, C, N), dtype=np.float32)
    for core in range(8):
        y[core // HEADS] += results[core]["y"]
    y += b_out.astype(np.float32)[None, :, None]
    return y.reshape(B, C, 16, 16, 16)


def run(x, w_qkv, w_out, b_out, trace=False):
    from concourse.bass_utils import run_bass_kernel_spmd
    nc = _get_nc()
    in_maps = _make_in_maps(np.asarray(x), np.asarray(w_qkv), np.asarray(w_out))
    res = run_bass_kernel_spmd(nc, in_maps, core_ids=list(range(8)),
                           trace=trace)
    return _gather(res.results, np.asarray(b_out)), res


def kernel(x, w_qkv, w_out, b_out):
    y, _ = run(x, w_qkv, w_out, b_out)
    return y



# revision 19
# speedup vs baseline: 1.4605x; 1.4605x over previous
"""Trainium2 Bass kernel for nn_Attention_42348377538911.

3D attention: x [2, 128, 16, 16, 16] -> qkv 1x1x1 conv -> 4-head attention
over N=4096 positions (dim_head=32) -> out 1x1x1 conv.

Sharding: 8 cores = 2 batches x 4 heads (one (b, h) pair per core).
Each core computes its head's attention and a tensor-parallel partial of the
output projection (w_out split along hidden); host sums the 4 partials per
batch and adds b_out.

Per-core layout (attention in simT = [j, i] orientation):
  qk-proj : psum[128, 512] = w_{q,k}T(x4 replicated).T @ x-tile; q evac ACT,
            k evac DVE (full [128,512] copies; any partition band holds any
            chunk thanks to the 4x replication, so slices are taken on use)
  simT    : 16 groups of 2 j-chunks per i-tile; qk psum pool [128, 1024]
            bufs=3 so QK(g+3) only waits exp(g) -- breaks the serial chain
  exp     : split between ACT (true exp -> bf16) and DVE (Schraudolph int16
            affine producing bf16 bits); softmax denominator comes free via a
            ones column in the AV moving operand
  AV      : flipped orientation: psum[128i, 33] += ex-chunk[128j,128i]
            (stationary) x vt_aug[128j, 33] (moving) -- output free size 33
            per matmul instead of 512, 4x fewer PE cycles than streaming P
  norm    : denominators land per-PARTITION -> DVE reciprocal + per-partition
            tensor_scalar multiply -> bf16
  y-proj  : DVE stream-transpose (32x32 blocks) then banded matmuls against
            4x-replicated w_oT -> psum [128c, 512i]; ACT evacuates; DMA out

PSUM: qk pool 3x2 banks + av 1 bank + y 1 bank = 8.
"""

import sys

import numpy as np

if "/opt/trn_rl_repo" not in sys.path:
    sys.path.insert(0, "/opt/trn_rl_repo")

HEADS = 4
DIM_HEAD = 32
B = 2
C = 128
N = 4096          # 16*16*16 spatial positions
NT = 512          # i-tile width
N_IT = N // NT    # 8 i-tiles
N_GRP = 16        # 2-chunk groups per i-tile
GW = 2            # chunks per group

# Schraudolph exp in bf16-bit domain: i16 = round(s * 128/ln2 + (127*128 - C))
SCH_A = 128.0 / float(np.log(2.0))
SCH_B = 127.0 * 128.0 - 5.0
# exp engine split: even groups on ACT (true exp), odd groups on DVE
# (Schraudolph); the last group is split at column ACT_TAIL
ACT_TAIL = 640

_cached = {}


def _build():
    import concourse.bacc as bacc
    import concourse.tile as tile
    import concourse.mybir as mybir
    from concourse.bass import ts

    f32 = mybir.dt.float32
    f32r = mybir.dt.float32r
    bf16 = mybir.dt.bfloat16
    i16 = mybir.dt.int16
    EXP = mybir.ActivationFunctionType.Exp
    COPY = mybir.ActivationFunctionType.Copy
    MULT = mybir.AluOpType.mult
    ADD = mybir.AluOpType.add

    nc = bacc.Bacc("TRN2", target_bir_lowering=False, debug=False, num_devices=8)
    x_d = nc.dram_tensor("x", [C, N], f32, kind="ExternalInput").ap()
    w4q_d = nc.dram_tensor("w_4q", [C, C], f32, kind="ExternalInput").ap()
    w4k_d = nc.dram_tensor("w_4k", [C, C], f32, kind="ExternalInput").ap()
    wvt_d = nc.dram_tensor("w_vT", [C, DIM_HEAD], f32, kind="ExternalInput").ap()
    wort_d = nc.dram_tensor("w_oT_z", [C, 4 * C], bf16, kind="ExternalInput").ap()
    y_d = nc.dram_tensor("y", [C, N], f32, kind="ExternalOutput").ap()

    GCOLS = GW * NT  # columns per qk psum group

    with tile.TileContext(nc) as tc:
        with tc.tile_pool(name="sing", bufs=1) as sing:
            w4q = sing.tile([C, C], f32r)
            w4k = sing.tile([C, C], f32r)
            wvt = sing.tile([C, DIM_HEAD], f32r)
            wort = sing.tile([C, 4, C], bf16)  # band-masked w_oT per pb
            x_sb = [sing.tile([C, NT], f32r, tag=f"x{cx}", name=f"x{cx}")
                    for cx in range(N_IT)]
            q_rt = [sing.tile([128, NT], f32r, tag=f"qrt{it}", name=f"qrt{it}")
                    for it in range(N_IT)]
            k_sb = [sing.tile([128, NT], f32r, tag=f"ksb{it}", name=f"ksb{it}")
                    for it in range(N_IT)]
            vt_aug = sing.tile([128, 32, 33], bf16)   # [j-in-chunk, chunk, d+1]
            scr = sing.tile([1, 64], f32)

            nc.sync.dma_start(w4q, w4q_d.bitcast(f32r))
            nc.sync.dma_start(w4k, w4k_d.bitcast(f32r))
            nc.sync.dma_start(wvt, wvt_d.bitcast(f32r))
            nc.sync.dma_start(
                wort[:], wort_d.rearrange("p (b c) -> p b c", c=C))
            # split x DMA issue across two queues so HWDGE generation overlaps
            for cx in range(N_IT):
                eng = nc.scalar if cx % 2 == 0 else nc.sync
                eng.dma_start(x_sb[cx], x_d[:, ts(cx, NT)].bitcast(f32r))
            # PE warmup fodder with no DMA dependency (ramps the p-state)
            wsrc = sing.tile([128, 128], f32r)
            nc.gpsimd.memset(wsrc[:].bitcast(f32), 0.0)
            # warm the ACT exp table while P0 runs
            nc.gpsimd.memset(scr, 0.0)
            nc.scalar.activation(scr, scr, EXP)
            nc.gpsimd.memset(vt_aug[:], 1.0)

            # ------- P0: replicated q/k projections -------
            with tc.tile_pool(name="p0ps", bufs=4, space="PSUM") as p0ps:
                warm = p0ps.tile([128, NT], f32, tag="pq")
                for _ in range(9):
                    nc.tensor.matmul(warm[:, 0:C], lhsT=wsrc, rhs=wsrc,
                                     start=True, stop=True)
                for it in range(N_IT):
                    psq = p0ps.tile([128, NT], f32, tag="pq")
                    nc.tensor.matmul(psq, lhsT=w4q, rhs=x_sb[it],
                                     start=True, stop=True)
                    nc.scalar.activation(q_rt[it], psq, COPY)
                    psk = p0ps.tile([128, NT], f32, tag="pk")
                    nc.tensor.matmul(psk, lhsT=w4k, rhs=x_sb[it],
                                     start=True, stop=True)
                    nc.vector.tensor_copy(k_sb[it], psk)

            # ---------------- P1: attention ----------------
            with tc.tile_pool(name="exp", bufs=32) as ex_pool, \
                 tc.tile_pool(name="nrm", bufs=3) as nrm, \
                 tc.tile_pool(name="ysb", bufs=2) as ysb, \
                 tc.tile_pool(name="qkp", bufs=3, space="PSUM") as qkp, \
                 tc.tile_pool(name="avp", bufs=1, space="PSUM") as avp, \
                 tc.tile_pool(name="yps", bufs=1, space="PSUM") as yps:

                ex_tiles = [None] * N_IT
                avnT_tiles = [None] * N_IT
                for step in range(N_IT + 2):
                    do_qk = step < N_IT
                    do_av = 1 <= step <= N_IT
                    do_y = step >= 2
                    cur = [] if do_qk else None
                    if do_av:
                        av_ps = avp.tile([128, 4 * 33], f32, tag="avy")
                        prev_ex = ex_tiles[step - 1]
                    if do_y:
                        avnT_y = avnT_tiles[step - 2]
                        y_ps = yps.tile([128, NT], f32, tag="yy")

                    # Interleave per group: QK(t) + exp, AV(t-1) chunk-major
                    # accumulate, and one y(t-2) matmul -- keeps the in-order
                    # PE queue fed while exp drains the qk psum slots.
                    for gk in range(N_GRP):
                        if do_qk:
                            jc0 = GW * gk
                            qk_ps = qkp.tile([128, GCOLS], f32, tag="qk")
                            ex_t = ex_pool.tile([128, GCOLS], bf16, tag="ex")
                            for r in range(GW):
                                jc = jc0 + r
                                nc.tensor.matmul(
                                    qk_ps[:, ts(r, NT)],
                                    lhsT=k_sb[jc // 4][32 * r:32 * r + 32,
                                                       ts(jc % 4, 128)],
                                    rhs=q_rt[step][32 * r:32 * r + 32, :],
                                    start=True, stop=True,
                                    tile_position=(32 * r, 0))
                            if gk == N_GRP - 1:
                                nc.scalar.activation(ex_t[:, 0:ACT_TAIL],
                                                     qk_ps[:, 0:ACT_TAIL], EXP)
                                nc.vector.tensor_scalar(
                                    ex_t[:, ACT_TAIL:GCOLS].bitcast(i16),
                                    qk_ps[:, ACT_TAIL:GCOLS],
                                    SCH_A, SCH_B, MULT, ADD)
                            elif gk % 2 == 0:
                                nc.scalar.activation(ex_t, qk_ps, EXP)
                            else:
                                nc.vector.tensor_scalar(
                                    ex_t[:, 0:GCOLS].bitcast(i16), qk_ps,
                                    SCH_A, SCH_B, MULT, ADD)
                            cur.append((ex_t, jc0))
                        if do_av:
                            # one av region (i-block) per 4 groups; the PE
                            # tolerates only ONE open accumulation chain, so
                            # each region's 32-matmul chain stays contiguous
                            # (self-contained QK/y matmuls between are fine)
                            ib = gk // 4
                            for m in range(8):
                                jc = (gk % 4) * 8 + m
                                ex_p, _ = prev_ex[jc // 2]
                                r = jc % 2
                                nc.tensor.matmul(
                                    av_ps[:, 33 * ib:33 * ib + 33],
                                    lhsT=ex_p[:, r * NT + ib * 128:
                                              r * NT + (ib + 1) * 128],
                                    rhs=vt_aug[:, jc, :],
                                    start=(jc == 0), stop=(jc == 31))
                        if do_y:
                            # band-masked stationary (zeros off-band) makes
                            # this a full-K matmul with no tile_position
                            pb, fb = gk % 4, gk // 4
                            nc.tensor.matmul(
                                y_ps[:, ts(gk, 32)],
                                lhsT=wort[:, pb, :],
                                rhs=avnT_y[:, ts(fb, 32)],
                                start=True, stop=True)
                    if do_qk:
                        ex_tiles[step] = cur

                    # ---- vT projection during step 0 (borrows av bank) ----
                    if step == 0:
                        for half in range(2):
                            ps2 = avp.tile([128, NT], f32, tag="avy")
                            for jj in range(16):
                                jc = half * 16 + jj
                                nc.tensor.matmul(
                                    ps2[:, ts(jj, 32)],
                                    lhsT=x_sb[jc // 4][:, ts(jc % 4, 128)],
                                    rhs=wvt,
                                    start=True, stop=True)
                            nc.vector.tensor_copy(
                                vt_aug[:, half * 16:(half + 1) * 16,
                                       0:DIM_HEAD],
                                ps2[:].rearrange("p (c d) -> p c d",
                                                 d=DIM_HEAD))

                    # ---- y evac + DMA for i-tile step-2 ----
                    if do_y:
                        it = step - 2
                        y_sb = ysb.tile([128, NT], f32, tag="ysb")
                        nc.scalar.activation(y_sb, y_ps, COPY)
                        nc.sync.dma_start(y_d[:, ts(it, NT)], y_sb)

                    # ---- normalize + transpose for i-tile step-1 ----
                    if do_av:
                        it = step - 1
                        ex_tiles[it] = None
                        av_v = av_ps[:].rearrange("p (b c) -> p b c", c=33)
                        rc = nrm.tile([128, 4], f32, tag="rc")
                        nc.vector.reciprocal(rc, av_v[:, :, 32])
                        avn = nrm.tile([128, 128], bf16, tag="avn")
                        nc.vector.tensor_tensor(
                            avn[:].rearrange("p (b c) -> p b c", c=DIM_HEAD),
                            av_v[:, :, 0:DIM_HEAD],
                            rc[:].unsqueeze(2).broadcast_to((128, 4, DIM_HEAD)),
                            MULT)
                        avnT = nrm.tile([128, 128], bf16, tag="avnT")
                        nc.vector.transpose(avnT, avn)
                        avnT_tiles[it] = avnT

    nc.compile()
    return nc


def _get_nc():
    if "nc" not in _cached:
        _cached["nc"] = _build()
    return _cached["nc"]


def _make_in_maps(x, w_qkv, w_out):
    import ml_dtypes

    scale = DIM_HEAD ** -0.5
    in_maps = []
    for core in range(8):
        b, h = core // HEADS, core % HEADS
        w_q = w_qkv[h * DIM_HEAD:(h + 1) * DIM_HEAD, :]
        w_k = w_qkv[128 + h * DIM_HEAD:128 + (h + 1) * DIM_HEAD, :]
        w_v = w_qkv[256 + h * DIM_HEAD:256 + (h + 1) * DIM_HEAD, :]
        w_oT = w_out[:, h * DIM_HEAD:(h + 1) * DIM_HEAD].T  # [d, c]
        # band-masked replicas: band pb of slot pb holds w_oT, rest zero
        wortz = np.zeros((C, 4, C), np.float32)
        for pb in range(4):
            wortz[32 * pb:32 * pb + 32, pb, :] = w_oT
        in_maps.append({
            "x": np.ascontiguousarray(x[b].reshape(C, N)),
            "w_4q": np.ascontiguousarray(np.tile(w_q.T * scale, (1, 4))),
            "w_4k": np.ascontiguousarray(np.tile(w_k.T, (1, 4))),
            "w_vT": np.ascontiguousarray(w_v.T),
            "w_oT_z": np.ascontiguousarray(
                wortz.reshape(C, 4 * C).astype(ml_dtypes.bfloat16)),
        })
    return in_maps


def _gather(results, b_out):
    y = np.zeros((B, C, N), dtype=np.float32)
    for core in range(8):
        y[core // HEADS] += results[core]["y"]
    y += b_out.astype(np.float32)[None, :, None]
    return y.reshape(B, C, 16, 16, 16)


def run(x, w_qkv, w_out, b_out, trace=False):
    from concourse.bass_utils import run_bass_kernel_spmd
    nc = _get_nc()
    in_maps = _make_in_maps(np.asarray(x), np.asarray(w_qkv), np.asarray(w_out))
    res = run_bass_kernel_spmd(nc, in_maps, core_ids=list(range(8)),
                               trace=trace)
    return _gather(res.results, np.asarray(b_out)), res


def kernel(x, w_qkv, w_out, b_out):
    y, _ = run(x, w_qkv, w_out, b_out)
    return y
